# revision 2
# baseline (speedup 1.0000x reference)
"""Contrastive loss kernel for Trainium2 (8 NeuronCores).

Strategy: shard the pairwise score computation on a 4x2 grid (4 caption
groups x 2 image groups).  Each core computes its block of the raw pairwise
dot tensor  g[i,w,j,r] = s[i,w,:] . im[j,r,:]  (the dominant 15 GFLOP
contraction over D=1024) on the TensorEngine in fp16 (1 cycle/row vs 4 for
fp32, half the HBM bytes).  Rows of the device matmul are (j,r) pairs --
1152 = 9*128 exactly, so no ragged m-tiles -- and columns are the (i,w)
pairs of the local caption group.  The remaining cheap reductions
(leaky-relu attention, softmax, top-k word pooling, entity-matched direct
score, margin reduction) run on host in float32.
"""

import os
import sys

import numpy as np

sys.path.insert(0, "/opt/trn_rl_repo")

B, R, L, D = 64, 36, 50, 1024
N_CORES = 8
TI, TJ = 4, 2                  # caption groups x image groups
BT_LOC = B // TI               # 16 captions per core
BI_LOC = B // TJ               # 32 images per core
M = BI_LOC * R                 # 1152 matmul rows   (j_local, r) = 9 * 128
N = BT_LOC * L                 # 800 matmul cols    (i_local, w)
K = D                          # 1024 contraction
NCH = 400                      # psum free-dim chunk (2 chunks of 400)
LAMBDA_SOFTMAX = 9.0
MARGIN = 0.2
EPS = 1e-8

_CACHE = {}
LAST_RESULTS = None  # BassKernelResults from the most recent run (for test.py)


def _build_bass():
    import concourse.bacc as bacc
    import concourse.mybir as mybir
    import concourse.tile as tile

    nc = bacc.Bacc(
        "TRN2",
        target_bir_lowering=False,
        debug=False,
        enable_asserts=False,
        num_devices=N_CORES,
    )
    f16 = mybir.dt.float16
    f32 = mybir.dt.float32
    sT = nc.dram_tensor("sT", [K, N], f16, kind="ExternalInput").ap()
    imT = nc.dram_tensor("imT", [K, M], f16, kind="ExternalInput").ap()
    gT = nc.dram_tensor("gT", [M, N], f16, kind="ExternalOutput").ap()

    KT = K // 128          # 8 contraction tiles
    MT = M // 128          # 9 row tiles (even)
    NT = N // NCH          # 2 column chunks
    with tile.TileContext(nc) as tc:
        with (
            tc.tile_pool(name="sp", bufs=KT * NT) as sp,
            tc.tile_pool(name="ip", bufs=KT * MT) as ip,
            tc.tile_pool(name="ps", bufs=6, space="PSUM") as pp,
            tc.tile_pool(name="out", bufs=6) as op,
        ):
            # Input DMAs, ordered so the first psum group's operands land
            # first: s column-half n=0 interleaved with im m-tile 0, then the
            # remaining im m-tiles, then the s n=1 half.
            sts = {}
            its = {}
            for ki in range(KT):
                st = sp.tile([128, NCH], f16, tag="s")
                nc.sync.dma_start(st[:], sT[ki * 128:(ki + 1) * 128, 0:NCH])
                sts[(ki, 0)] = st
                it = ip.tile([128, 128], f16, tag="im")
                nc.sync.dma_start(it[:], imT[ki * 128:(ki + 1) * 128, 0:128])
                its[(ki, 0)] = it
            for mi in range(1, MT):
                for ki in range(KT):
                    it = ip.tile([128, 128], f16, tag="im")
                    nc.sync.dma_start(
                        it[:], imT[ki * 128:(ki + 1) * 128,
                                   mi * 128:(mi + 1) * 128])
                    its[(ki, mi)] = it
            for ki in range(KT):
                st = sp.tile([128, NCH], f16, tag="s")
                nc.sync.dma_start(
                    st[:], sT[ki * 128:(ki + 1) * 128, NCH:2 * NCH])
                sts[(ki, 1)] = st

            for ni in range(NT):
                for mi in range(MT):
                    ps = pp.tile([128, NCH], f32, tag="ps")
                    for ki in range(KT):
                        nc.tensor.matmul(
                            ps[:],
                            its[(ki, mi)][:],
                            sts[(ki, ni)][:],
                            start=(ki == 0),
                            stop=(ki == KT - 1),
                        )
                    ot = op.tile([128, NCH], f16, tag="out")
                    nc.vector.tensor_copy(ot[:], ps[:])
                    nc.sync.dma_start(
                        gT[mi * 128:(mi + 1) * 128, ni * NCH:(ni + 1) * NCH],
                        ot[:])
    nc.compile()
    return nc


def _run_device(s_np, im_np):
    """Returns g4 [B, B, L, R] fp32: g4[i,j,w,r] = s[i,w] . im[j,r]."""
    global LAST_RESULTS
    from concourse import bass_utils

    if "nc" not in _CACHE:
        _CACHE["nc"] = _build_bass()
    nc = _CACHE["nc"]

    s16 = s_np.astype(np.float16)
    im16 = im_np.astype(np.float16)
    in_maps = []
    for c in range(N_CORES):
        ti, tj = c // TJ, c % TJ
        sblk = s16[ti * BT_LOC:(ti + 1) * BT_LOC].reshape(N, K)
        iblk = im16[tj * BI_LOC:(tj + 1) * BI_LOC].reshape(M, K)
        in_maps.append({
            "sT": np.ascontiguousarray(sblk.T),    # [1024, 800]
            "imT": np.ascontiguousarray(iblk.T),   # [1024, 1152]
        })
    res = bass_utils.run_bass_kernel_spmd(
        nc, in_maps, core_ids=list(range(N_CORES)),
        trace=bool(os.environ.get("KERNEL_TRACE")),
    )
    LAST_RESULTS = res
    g4 = np.empty((B, B, L, R), dtype=np.float32)
    for c in range(N_CORES):
        ti, tj = c // TJ, c % TJ
        gb = res.results[c]["gT"].astype(np.float32)       # [1152, 800]
        blk = gb.reshape(BI_LOC, R, BT_LOC, L).transpose(2, 0, 3, 1)
        g4[ti * BT_LOC:(ti + 1) * BT_LOC,
           tj * BI_LOC:(tj + 1) * BI_LOC] = blk
    return g4


def _host_finish(g4, im, s, img_ent, cap_ent, cap_lens):
    f32 = np.float32
    w_idx = np.arange(L)
    word_valid = w_idx[None, :] < cap_lens[:, None]             # [Bt, L]

    attn = np.where(g4 > 0, g4, f32(0.1) * g4)
    attn = attn * word_valid[:, None, :, None].astype(f32)
    attn = attn / (np.sqrt(np.sum(attn * attn, axis=2, keepdims=True)) + f32(EPS))
    z = attn * f32(LAMBDA_SOFTMAX)
    z = z - z.max(axis=-1, keepdims=True)
    e = np.exp(z)
    a = e / e.sum(axis=-1, keepdims=True)
    a = a * (a > 1.0 / R).astype(f32)

    dot_swc = np.sum(a * g4, axis=-1)                           # [Bt,Bi,L]
    gram = np.einsum("jrd,jqd->jrq", im, im)                    # [Bi,R,R]
    t = np.einsum("ijwr,jrq->ijwq", a, gram, optimize=True)
    wc_sq = np.sum(t * a, axis=-1)
    wc_norm = np.sqrt(np.maximum(wc_sq, f32(1e-24)))
    ns = np.sqrt(np.sum(s * s, axis=-1))                        # [Bt,L]
    cos = dot_swc / np.maximum(ns[:, None, :] * wc_norm, f32(EPS))
    cos = np.where(word_valid[:, None, :], cos, f32(-np.inf))
    srt = np.sort(cos, axis=-1)[..., ::-1]
    k = cap_lens - cap_lens // 3
    keep = w_idx[None, None, :] < k[:, None, None]
    latent = np.where(keep, srt, f32(0.0)).sum(axis=-1) / k[:, None].astype(f32)

    n_min = np.minimum(cap_lens, 50)
    ent_ok = (cap_ent != 0) & (w_idx[None, :] < n_min[:, None])
    match = (cap_ent[:, None, :, None] == img_ent[None, :, None, :]) \
        & ent_ok[:, None, :, None]
    nim = np.sqrt(np.sum(im * im, axis=-1))                     # [Bi,R]
    denom = np.maximum(ns[:, None, :, None] * nim[None, :, None, :], f32(EPS))
    direct = np.where(match, g4 / denom, f32(0.0)).sum(axis=(2, 3)) \
        / n_min[:, None].astype(f32)

    scores = latent + direct                                    # [Bt,Bi]
    diag = np.diag(scores).copy()
    cost_s = np.maximum(f32(MARGIN) + scores - diag[:, None], f32(0.0))
    cost_im = np.maximum(f32(MARGIN) + scores - diag[None, :], f32(0.0))
    np.fill_diagonal(cost_s, 0.0)
    np.fill_diagonal(cost_im, 0.0)
    return np.float32(cost_s.max(axis=1).sum() + cost_im.max(axis=0).sum())


def kernel(im, s, image_entity_idxs, caps_entity_idxs, s_l):
    im = np.asarray(im, dtype=np.float32)
    s = np.asarray(s, dtype=np.float32)
    img_ent = np.asarray(image_entity_idxs)
    cap_ent = np.asarray(caps_entity_idxs)
    cap_lens = np.asarray(s_l)
    g4 = _run_device(s, im)
    return _host_finish(g4, im, s, img_ent, cap_ent, cap_lens)


# revision 5
# speedup vs baseline: 1.9141x; 1.9141x over previous
"""Contrastive loss kernel for Trainium2 (8 NeuronCores).

Strategy: shard the pairwise score computation on a 4x2 grid (4 caption
groups x 2 image groups).  Each core computes its block of the raw pairwise
dot tensor  g[i,w,j,r] = s[i,w,:] . im[j,r,:]  (the dominant 15 GFLOP
contraction over D=1024) on the TensorEngine in fp16 (1 cycle/row vs 4 for
fp32, half the HBM bytes).  Rows of the device matmul are (j,r) pairs --
1152 = 9*128 exactly, so no ragged m-tiles -- and columns are the (i,w)
pairs of the local caption group.

DMA plan: few large transfers (descriptor generation on the Sync engine
costs ~650 ns per dma_start, so many small DMAs serialize the kernel).
im k-tiles go on the sync HWDGE ring, s k-tiles on the scalar HWDGE ring,
outputs on the gpsimd SWDGE ring.  The first 8 psum groups accumulate
k-outer (one k-step per group as each (im_k, s_k) pair lands) so the PE
never waits for the full input set.

The remaining cheap reductions (leaky-relu attention, softmax, top-k word
pooling, entity-matched direct score, margin reduction) run on host in
float32.
"""

import os
import sys

import numpy as np

sys.path.insert(0, "/opt/trn_rl_repo")

B, R, L, D = 64, 36, 50, 1024
N_CORES = 8
TI, TJ = 4, 2                  # caption groups x image groups
BT_LOC = B // TI               # 16 captions per core
BI_LOC = B // TJ               # 32 images per core
M = BI_LOC * R                 # 1152 matmul rows   (j_local, r) = 9 * 128
N = BT_LOC * L                 # 800 matmul cols    (i_local, w)
K = D                          # 1024 contraction
NCH = 400                      # psum free-dim chunk (2 chunks of 400)
MA = 4                         # m-tiles in wave 1 (im column chunk A: 512)
LAMBDA_SOFTMAX = 9.0
MARGIN = 0.2
EPS = 1e-8

_CACHE = {}
LAST_RESULTS = None  # BassKernelResults from the most recent run (for test.py)


def _build_bass():
    import concourse.bacc as bacc
    import concourse.mybir as mybir
    import concourse.tile as tile

    nc = bacc.Bacc(
        "TRN2",
        target_bir_lowering=False,
        debug=False,
        enable_asserts=False,
        num_devices=N_CORES,
    )
    f16 = mybir.dt.float16
    f32 = mybir.dt.float32
    sT = nc.dram_tensor("sT", [K, N], f16, kind="ExternalInput").ap()
    imT = nc.dram_tensor("imT", [K, M], f16, kind="ExternalInput").ap()
    gT = nc.dram_tensor("gT", [M, N], f16, kind="ExternalOutput").ap()

    KT = K // 128          # 8 contraction tiles
    MT = M // 128          # 9 row tiles (even)
    NT = N // NCH          # 2 column chunks
    CA = MA * 128          # 512 columns of im in chunk A
    with tile.TileContext(nc) as tc:
        with (
            tc.tile_pool(name="sp", bufs=KT) as sp,
            tc.tile_pool(name="ia", bufs=KT) as iap,
            tc.tile_pool(name="ib", bufs=KT) as ibp,
            tc.tile_pool(name="ps", bufs=8, space="PSUM") as pp,
            tc.tile_pool(name="out", bufs=4) as op,
        ):
            # Wave-1 inputs: im chunk A (m-tiles 0..3) on sync ring, s on
            # scalar ring, interleaved k-wise so pair k lands early.
            ias, sts = [], []
            for ki in range(KT):
                it = iap.tile([128, CA], f16, tag="ia")
                nc.sync.dma_start(it[:], imT[ki * 128:(ki + 1) * 128, 0:CA])
                ias.append(it)
                st = sp.tile([128, N], f16, tag="s")
                nc.scalar.dma_start(st[:], sT[ki * 128:(ki + 1) * 128, :])
                sts.append(st)
            # Wave-2 inputs: im chunk B (m-tiles 4..8).
            ibs = []
            for ki in range(KT):
                it = ibp.tile([128, M - CA], f16, tag="ib")
                nc.sync.dma_start(it[:], imT[ki * 128:(ki + 1) * 128, CA:M])
                ibs.append(it)

            outs = {}

            def finish_group(mi, ni, ps):
                if mi not in outs:
                    outs[mi] = op.tile([128, N], f16, tag="out",
                                       name=f"out_{mi}")
                ot = outs[mi]
                nc.vector.tensor_copy(ot[:, ni * NCH:(ni + 1) * NCH], ps[:])
                if ni == NT - 1:
                    nc.gpsimd.dma_start(
                        gT[mi * 128:(mi + 1) * 128, :], ot[:])

            # Wave 1: 8 psum groups (m 0..3 x n 0..1), k-outer so each
            # (im_k, s_k) arrival unlocks one k-step for every open group.
            w1 = [(mi, ni) for mi in range(MA) for ni in range(NT)]
            pss = {g: pp.tile([128, NCH], f32, tag="ps", name=f"ps_{g[0]}_{g[1]}")
                   for g in w1}
            for ki in range(KT):
                for (mi, ni) in w1:
                    nc.tensor.matmul(
                        pss[(mi, ni)][:],
                        ias[ki][:, mi * 128:(mi + 1) * 128],
                        sts[ki][:, ni * NCH:(ni + 1) * NCH],
                        start=(ki == 0),
                        stop=(ki == KT - 1),
                    )
            for (mi, ni) in w1:
                finish_group(mi, ni, pss[(mi, ni)])

            # Wave 2: m-tiles 4..8, data resident by now; k-inner per group.
            for mi in range(MA, MT):
                for ni in range(NT):
                    ps = pp.tile([128, NCH], f32, tag="ps")
                    for ki in range(KT):
                        nc.tensor.matmul(
                            ps[:],
                            ibs[ki][:, (mi - MA) * 128:(mi - MA + 1) * 128],
                            sts[ki][:, ni * NCH:(ni + 1) * NCH],
                            start=(ki == 0),
                            stop=(ki == KT - 1),
                        )
                    finish_group(mi, ni, ps)
    nc.compile()
    return nc


def _run_device(s_np, im_np):
    """Returns g4 [B, B, L, R] fp32: g4[i,j,w,r] = s[i,w] . im[j,r]."""
    global LAST_RESULTS
    from concourse import bass_utils

    if "nc" not in _CACHE:
        _CACHE["nc"] = _build_bass()
    nc = _CACHE["nc"]

    s16 = s_np.astype(np.float16)
    im16 = im_np.astype(np.float16)
    in_maps = []
    for c in range(N_CORES):
        ti, tj = c // TJ, c % TJ
        sblk = s16[ti * BT_LOC:(ti + 1) * BT_LOC].reshape(N, K)
        iblk = im16[tj * BI_LOC:(tj + 1) * BI_LOC].reshape(M, K)
        in_maps.append({
            "sT": np.ascontiguousarray(sblk.T),    # [1024, 800]
            "imT": np.ascontiguousarray(iblk.T),   # [1024, 1152]
        })
    res = bass_utils.run_bass_kernel_spmd(
        nc, in_maps, core_ids=list(range(N_CORES)),
        trace=bool(os.environ.get("KERNEL_TRACE")),
    )
    LAST_RESULTS = res
    g4 = np.empty((B, B, L, R), dtype=np.float32)
    for c in range(N_CORES):
        ti, tj = c // TJ, c % TJ
        gb = res.results[c]["gT"].astype(np.float32)       # [1152, 800]
        blk = gb.reshape(BI_LOC, R, BT_LOC, L).transpose(2, 0, 3, 1)
        g4[ti * BT_LOC:(ti + 1) * BT_LOC,
           tj * BI_LOC:(tj + 1) * BI_LOC] = blk
    return g4


def _host_finish(g4, im, s, img_ent, cap_ent, cap_lens):
    f32 = np.float32
    w_idx = np.arange(L)
    word_valid = w_idx[None, :] < cap_lens[:, None]             # [Bt, L]

    attn = np.where(g4 > 0, g4, f32(0.1) * g4)
    attn = attn * word_valid[:, None, :, None].astype(f32)
    attn = attn / (np.sqrt(np.sum(attn * attn, axis=2, keepdims=True)) + f32(EPS))
    z = attn * f32(LAMBDA_SOFTMAX)
    z = z - z.max(axis=-1, keepdims=True)
    e = np.exp(z)
    a = e / e.sum(axis=-1, keepdims=True)
    a = a * (a > 1.0 / R).astype(f32)

    dot_swc = np.sum(a * g4, axis=-1)                           # [Bt,Bi,L]
    gram = np.einsum("jrd,jqd->jrq", im, im)                    # [Bi,R,R]
    t = np.einsum("ijwr,jrq->ijwq", a, gram, optimize=True)
    wc_sq = np.sum(t * a, axis=-1)
    wc_norm = np.sqrt(np.maximum(wc_sq, f32(1e-24)))
    ns = np.sqrt(np.sum(s * s, axis=-1))                        # [Bt,L]
    cos = dot_swc / np.maximum(ns[:, None, :] * wc_norm, f32(EPS))
    cos = np.where(word_valid[:, None, :], cos, f32(-np.inf))
    srt = np.sort(cos, axis=-1)[..., ::-1]
    k = cap_lens - cap_lens // 3
    keep = w_idx[None, None, :] < k[:, None, None]
    latent = np.where(keep, srt, f32(0.0)).sum(axis=-1) / k[:, None].astype(f32)

    n_min = np.minimum(cap_lens, 50)
    ent_ok = (cap_ent != 0) & (w_idx[None, :] < n_min[:, None])
    match = (cap_ent[:, None, :, None] == img_ent[None, :, None, :]) \
        & ent_ok[:, None, :, None]
    nim = np.sqrt(np.sum(im * im, axis=-1))                     # [Bi,R]
    denom = np.maximum(ns[:, None, :, None] * nim[None, :, None, :], f32(EPS))
    direct = np.where(match, g4 / denom, f32(0.0)).sum(axis=(2, 3)) \
        / n_min[:, None].astype(f32)

    scores = latent + direct                                    # [Bt,Bi]
    diag = np.diag(scores).copy()
    cost_s = np.maximum(f32(MARGIN) + scores - diag[:, None], f32(0.0))
    cost_im = np.maximum(f32(MARGIN) + scores - diag[None, :], f32(0.0))
    np.fill_diagonal(cost_s, 0.0)
    np.fill_diagonal(cost_im, 0.0)
    return np.float32(cost_s.max(axis=1).sum() + cost_im.max(axis=0).sum())


def kernel(im, s, image_entity_idxs, caps_entity_idxs, s_l):
    im = np.asarray(im, dtype=np.float32)
    s = np.asarray(s, dtype=np.float32)
    img_ent = np.asarray(image_entity_idxs)
    cap_ent = np.asarray(caps_entity_idxs)
    cap_lens = np.asarray(s_l)
    g4 = _run_device(s, im)
    return _host_finish(g4, im, s, img_ent, cap_ent, cap_lens)


# revision 6
# speedup vs baseline: 2.5350x; 1.3243x over previous
"""Contrastive loss kernel for Trainium2 (8 NeuronCores).

Strategy: shard the pairwise score computation on a 4x2 grid (4 caption
groups x 2 image groups).  Each core computes its block of the raw pairwise
dot tensor  g[i,w,j,r] = s[i,w,:] . im[j,r,:]  (the dominant 15 GFLOP
contraction over D=1024) on the TensorEngine in fp8-e4m3 with DoubleRow
perf mode (2 MACs/cell/cycle).  Rows of the device matmul are (j,r) pairs
-- 1152 = 9*128 exactly, so no ragged m-tiles -- and columns are the (i,w)
pairs of the local caption group.  PSUM accumulates in fp32 and the g block
is written back in fp16, so the only precision loss is the fp8 input
rounding (~1e-3 on the final loss; tolerance is 2e-2).

DMA plan: few large transfers (descriptor generation costs ~650 ns per
dma_start on the issuing sequencer).  im k-group tiles go on the sync
HWDGE ring, s k-group tiles on the scalar HWDGE ring, outputs on the
gpsimd SWDGE ring.  The first 8 psum groups accumulate k-outer (one k-step
per group as each (im_kg, s_kg) pair lands) so the PE never waits for the
full input set.

The remaining cheap reductions (leaky-relu attention, softmax, top-k word
pooling, entity-matched direct score, margin reduction) run on host in
float32.
"""

import os
import sys

import numpy as np

sys.path.insert(0, "/opt/trn_rl_repo")

B, R, L, D = 64, 36, 50, 1024
N_CORES = 8
TI, TJ = 4, 2                  # caption groups x image groups
BT_LOC = B // TI               # 16 captions per core
BI_LOC = B // TJ               # 32 images per core
M = BI_LOC * R                 # 1152 matmul rows   (j_local, r) = 9 * 128
N = BT_LOC * L                 # 800 matmul cols    (i_local, w)
K = D                          # 1024 contraction
KG = 4                         # k-groups of 256 (2 x 128 for DoubleRow)
NCH = 400                      # psum free-dim chunk (2 chunks of 400)
MA = 4                         # m-tiles in wave 1
LAMBDA_SOFTMAX = 9.0
MARGIN = 0.2
EPS = 1e-8

_CACHE = {}
LAST_RESULTS = None  # BassKernelResults from the most recent run (for test.py)


def _build_bass():
    import concourse.bacc as bacc
    import concourse.mybir as mybir
    import concourse.tile as tile

    nc = bacc.Bacc(
        "TRN2",
        target_bir_lowering=False,
        debug=False,
        enable_asserts=False,
        num_devices=1,
    )
    f8 = mybir.dt.float8e4
    f16 = mybir.dt.float16
    f32 = mybir.dt.float32
    dr = mybir.MatmulPerfMode.DoubleRow
    # [kg, p, i, c]: element = xT[kg*256 + i*128 + p, c]
    sT8 = nc.dram_tensor("sT8", [KG, 128, 2, N], f8, kind="ExternalInput").ap()
    imT8 = nc.dram_tensor("imT8", [KG, 128, 2, M], f8,
                          kind="ExternalInput").ap()
    gT = nc.dram_tensor("gT", [M, N], f16, kind="ExternalOutput").ap()

    MT = M // 128          # 9 row tiles (even)
    NT = N // NCH          # 2 column chunks
    with tile.TileContext(nc) as tc:
        with (
            tc.tile_pool(name="sp", bufs=KG) as sp,
            tc.tile_pool(name="ip", bufs=KG) as ip,
            tc.tile_pool(name="ps", bufs=8, space="PSUM") as pp,
            tc.tile_pool(name="out", bufs=4) as op,
        ):
            ims, sts = [], []
            for kg in range(KG):
                it = ip.tile([128, 2, M], f8, tag="im", name=f"im_{kg}")
                nc.sync.dma_start(it[:], imT8[kg])
                ims.append(it)
                st = sp.tile([128, 2, N], f8, tag="s", name=f"s_{kg}")
                nc.scalar.dma_start(st[:], sT8[kg])
                sts.append(st)

            outs = {}

            def finish_group(mi, ni, ps):
                if mi not in outs:
                    outs[mi] = op.tile([128, N], f16, tag="out",
                                       name=f"out_{mi}")
                ot = outs[mi]
                nc.vector.tensor_copy(ot[:, ni * NCH:(ni + 1) * NCH], ps[:])
                if ni == NT - 1:
                    nc.gpsimd.dma_start(
                        gT[mi * 128:(mi + 1) * 128, :], ot[:])

            # Wave 1: 8 psum groups (m 0..3 x n 0..1), k-outer so each
            # (im_kg, s_kg) arrival unlocks one k-step for every open group.
            w1 = [(mi, ni) for mi in range(MA) for ni in range(NT)]
            pss = {g: pp.tile([128, NCH], f32, tag="ps", name=f"ps_{g[0]}_{g[1]}")
                   for g in w1}
            for kg in range(KG):
                for (mi, ni) in w1:
                    nc.tensor.matmul(
                        pss[(mi, ni)][:],
                        ims[kg][:, :, mi * 128:(mi + 1) * 128],
                        sts[kg][:, :, ni * NCH:(ni + 1) * NCH],
                        start=(kg == 0),
                        stop=(kg == KG - 1),
                        perf_mode=dr,
                    )
            for (mi, ni) in w1:
                finish_group(mi, ni, pss[(mi, ni)])

            # Wave 2: m-tiles 4..8, data resident by now; k-inner per group.
            for mi in range(MA, MT):
                for ni in range(NT):
                    ps = pp.tile([128, NCH], f32, tag="ps")
                    for kg in range(KG):
                        nc.tensor.matmul(
                            ps[:],
                            ims[kg][:, :, mi * 128:(mi + 1) * 128],
                            sts[kg][:, :, ni * NCH:(ni + 1) * NCH],
                            start=(kg == 0),
                            stop=(kg == KG - 1),
                            perf_mode=dr,
                        )
                    finish_group(mi, ni, ps)
    nc.compile()
    return nc


def _pack_fp8(xT):
    """xT [1024, C] fp8 -> [KG, 128, 2, C] with [kg,p,i,c] = xT[kg*256+i*128+p, c]."""
    C = xT.shape[1]
    return np.ascontiguousarray(
        xT.reshape(KG, 2, 128, C).transpose(0, 2, 1, 3))


def _run_device(s_np, im_np):
    """Returns g4 [B, B, L, R] fp32: g4[i,j,w,r] = s[i,w] . im[j,r]."""
    global LAST_RESULTS
    import ml_dtypes
    from concourse import bass_utils

    if "nc" not in _CACHE:
        _CACHE["nc"] = _build_bass()
    nc = _CACHE["nc"]

    f8 = ml_dtypes.float8_e4m3
    s8 = s_np.astype(f8)
    im8 = im_np.astype(f8)
    in_maps = []
    for c in range(N_CORES):
        ti, tj = c // TJ, c % TJ
        sblk = s8[ti * BT_LOC:(ti + 1) * BT_LOC].reshape(N, K)
        iblk = im8[tj * BI_LOC:(tj + 1) * BI_LOC].reshape(M, K)
        in_maps.append({
            "sT8": _pack_fp8(np.ascontiguousarray(sblk.T)),
            "imT8": _pack_fp8(np.ascontiguousarray(iblk.T)),
        })
    res = bass_utils.run_bass_kernel_spmd(
        nc, in_maps, core_ids=list(range(N_CORES)),
        trace=bool(os.environ.get("KERNEL_TRACE")),
    )
    LAST_RESULTS = res
    g4 = np.empty((B, B, L, R), dtype=np.float32)
    for c in range(N_CORES):
        ti, tj = c // TJ, c % TJ
        gb = res.results[c]["gT"].astype(np.float32)       # [1152, 800]
        blk = gb.reshape(BI_LOC, R, BT_LOC, L).transpose(2, 0, 3, 1)
        g4[ti * BT_LOC:(ti + 1) * BT_LOC,
           tj * BI_LOC:(tj + 1) * BI_LOC] = blk
    return g4


def _host_finish(g4, im, s, img_ent, cap_ent, cap_lens):
    f32 = np.float32
    w_idx = np.arange(L)
    word_valid = w_idx[None, :] < cap_lens[:, None]             # [Bt, L]

    attn = np.where(g4 > 0, g4, f32(0.1) * g4)
    attn = attn * word_valid[:, None, :, None].astype(f32)
    attn = attn / (np.sqrt(np.sum(attn * attn, axis=2, keepdims=True)) + f32(EPS))
    z = attn * f32(LAMBDA_SOFTMAX)
    z = z - z.max(axis=-1, keepdims=True)
    e = np.exp(z)
    a = e / e.sum(axis=-1, keepdims=True)
    a = a * (a > 1.0 / R).astype(f32)

    dot_swc = np.sum(a * g4, axis=-1)                           # [Bt,Bi,L]
    gram = np.einsum("jrd,jqd->jrq", im, im)                    # [Bi,R,R]
    t = np.einsum("ijwr,jrq->ijwq", a, gram, optimize=True)
    wc_sq = np.sum(t * a, axis=-1)
    wc_norm = np.sqrt(np.maximum(wc_sq, f32(1e-24)))
    ns = np.sqrt(np.sum(s * s, axis=-1))                        # [Bt,L]
    cos = dot_swc / np.maximum(ns[:, None, :] * wc_norm, f32(EPS))
    cos = np.where(word_valid[:, None, :], cos, f32(-np.inf))
    srt = np.sort(cos, axis=-1)[..., ::-1]
    k = cap_lens - cap_lens // 3
    keep = w_idx[None, None, :] < k[:, None, None]
    latent = np.where(keep, srt, f32(0.0)).sum(axis=-1) / k[:, None].astype(f32)

    n_min = np.minimum(cap_lens, 50)
    ent_ok = (cap_ent != 0) & (w_idx[None, :] < n_min[:, None])
    match = (cap_ent[:, None, :, None] == img_ent[None, :, None, :]) \
        & ent_ok[:, None, :, None]
    nim = np.sqrt(np.sum(im * im, axis=-1))                     # [Bi,R]
    denom = np.maximum(ns[:, None, :, None] * nim[None, :, None, :], f32(EPS))
    direct = np.where(match, g4 / denom, f32(0.0)).sum(axis=(2, 3)) \
        / n_min[:, None].astype(f32)

    scores = latent + direct                                    # [Bt,Bi]
    diag = np.diag(scores).copy()
    cost_s = np.maximum(f32(MARGIN) + scores - diag[:, None], f32(0.0))
    cost_im = np.maximum(f32(MARGIN) + scores - diag[None, :], f32(0.0))
    np.fill_diagonal(cost_s, 0.0)
    np.fill_diagonal(cost_im, 0.0)
    return np.float32(cost_s.max(axis=1).sum() + cost_im.max(axis=0).sum())


def kernel(im, s, image_entity_idxs, caps_entity_idxs, s_l):
    im = np.asarray(im, dtype=np.float32)
    s = np.asarray(s, dtype=np.float32)
    img_ent = np.asarray(image_entity_idxs)
    cap_ent = np.asarray(caps_entity_idxs)
    cap_lens = np.asarray(s_l)
    g4 = _run_device(s, im)
    return _host_finish(g4, im, s, img_ent, cap_ent, cap_lens)


# revision 8
# speedup vs baseline: 2.6321x; 1.0383x over previous
"""Contrastive loss kernel for Trainium2 (8 NeuronCores).

Strategy: shard the pairwise score computation on a 4x2 grid (4 caption
groups x 2 image groups).  Each core computes its block of the raw pairwise
dot tensor  g[i,w,j,r] = s[i,w,:] . im[j,r,:]  (the dominant 15 GFLOP
contraction over D=1024) on the TensorEngine in fp8-e4m3 with DoubleRow
perf mode (2 MACs/cell/cycle).  Rows of the device matmul are (j,r) pairs
-- 1152 = 9*128 exactly, so no ragged m-tiles -- and columns are the (i,w)
pairs of the local caption group.  PSUM accumulates in fp32 and the g block
is written back in fp16, so the only precision loss is the fp8 input
rounding (~1e-3 on the final loss; tolerance is 2e-2).

DMA plan: few large transfers (descriptor generation costs ~650 ns per
dma_start on the issuing sequencer).  im k-group tiles go on the sync
HWDGE ring, s k-group tiles on the scalar HWDGE ring, outputs on the
gpsimd SWDGE ring.  The first 8 psum groups accumulate k-outer (one k-step
per group as each (im_kg, s_kg) pair lands) so the PE never waits for the
full input set.

The remaining cheap reductions (leaky-relu attention, softmax, top-k word
pooling, entity-matched direct score, margin reduction) run on host in
float32.
"""

import os
import sys

import numpy as np

sys.path.insert(0, "/opt/trn_rl_repo")

B, R, L, D = 64, 36, 50, 1024
N_CORES = 8
TI, TJ = 4, 2                  # caption groups x image groups
BT_LOC = B // TI               # 16 captions per core
BI_LOC = B // TJ               # 32 images per core
M = BI_LOC * R                 # 1152 matmul rows   (j_local, r) = 9 * 128
N = BT_LOC * L                 # 800 matmul cols    (i_local, w)
K = D                          # 1024 contraction
KG = 4                         # k-groups of 256 (2 x 128 for DoubleRow)
NCH = 400                      # psum free-dim chunk (2 chunks of 400)
MA = 4                         # m-tiles in wave 1
LAMBDA_SOFTMAX = 9.0
MARGIN = 0.2
EPS = 1e-8

_CACHE = {}
LAST_RESULTS = None  # BassKernelResults from the most recent run (for test.py)


def _build_bass():
    import concourse.bacc as bacc
    import concourse.mybir as mybir
    import concourse.tile as tile

    nc = bacc.Bacc(
        "TRN2",
        target_bir_lowering=False,
        debug=False,
        enable_asserts=False,
        num_devices=1,
    )
    f8 = mybir.dt.float8e4
    f16 = mybir.dt.float16
    f32 = mybir.dt.float32
    dr = mybir.MatmulPerfMode.DoubleRow
    # [kg, p, i, c]: element = xT[kg*256 + i*128 + p, c]
    sT8 = nc.dram_tensor("sT8", [KG, 128, 2, N], f8, kind="ExternalInput").ap()
    imT8 = nc.dram_tensor("imT8", [KG, 128, 2, M], f8,
                          kind="ExternalInput").ap()
    gT = nc.dram_tensor("gT", [M, N], f16, kind="ExternalOutput").ap()

    MT = M // 128          # 9 row tiles (even)
    NT = N // NCH          # 2 column chunks
    with tile.TileContext(nc) as tc:
        with (
            tc.tile_pool(name="sp", bufs=KG) as sp,
            tc.tile_pool(name="ip", bufs=KG) as ip,
            tc.tile_pool(name="ps", bufs=8, space="PSUM") as pp,
            tc.tile_pool(name="out", bufs=4) as op,
            tc.tile_pool(name="wu", bufs=1) as wp,
        ):
            ims, sts = [], []
            for kg in range(KG):
                it = ip.tile([128, 2, M], f8, tag="im", name=f"im_{kg}")
                nc.sync.dma_start(it[:], imT8[kg])
                ims.append(it)
                st = sp.tile([128, 2, N], f8, tag="s", name=f"s_{kg}")
                nc.scalar.dma_start(st[:], sT8[kg])
                sts.append(st)

            outs = {}

            def finish_group(mi, ni, ps):
                if mi not in outs:
                    outs[mi] = op.tile([128, N], f16, tag="out",
                                       name=f"out_{mi}")
                ot = outs[mi]
                nc.vector.tensor_copy(ot[:, ni * NCH:(ni + 1) * NCH], ps[:])
                if mi == MT - 1:
                    # tail: ship each half as soon as its cast lands, on the
                    # idle scalar HWDGE ring (lower completion latency).
                    nc.scalar.dma_start(
                        gT[mi * 128:(mi + 1) * 128,
                           ni * NCH:(ni + 1) * NCH],
                        ot[:, ni * NCH:(ni + 1) * NCH])
                elif ni == NT - 1:
                    nc.gpsimd.dma_start(
                        gT[mi * 128:(mi + 1) * 128, :], ot[:])

            # Wave 1: 8 psum groups (m 0..3 x n 0..1), k-outer so each
            # (im_kg, s_kg) arrival unlocks one k-step for every open group.
            w1 = [(mi, ni) for mi in range(MA) for ni in range(NT)]
            pss = {g: pp.tile([128, NCH], f32, tag="ps", name=f"ps_{g[0]}_{g[1]}")
                   for g in w1}

            # HAM warm-up: the PE clock-gate needs ~3.4 us of sustained
            # activity to go 1.2 -> 2.4 GHz, and the input DMAs don't land
            # until ~4 us after engine init.  Fill that window with dummy
            # matmuls on a scratch tile; the scratch psum tile shares the
            # "ps" slots, so it is released before the last wave-1 group
            # needs its bank.
            wut = wp.tile([128, NCH], f16, tag="wu")
            nc.vector.memset(wut[:], 0.0)
            wups = pp.tile([128, NCH], f32, tag="ps", name="wups")
            for _ in range(12):
                nc.tensor.matmul(
                    wups[:], wut[:, 0:128], wut[:],
                    start=True, stop=True)

            for kg in range(KG):
                for (mi, ni) in w1:
                    nc.tensor.matmul(
                        pss[(mi, ni)][:],
                        ims[kg][:, :, mi * 128:(mi + 1) * 128],
                        sts[kg][:, :, ni * NCH:(ni + 1) * NCH],
                        start=(kg == 0),
                        stop=(kg == KG - 1),
                        perf_mode=dr,
                    )
            for (mi, ni) in w1:
                finish_group(mi, ni, pss[(mi, ni)])

            # Wave 2: m-tiles 4..8, data resident by now; k-inner per group.
            for mi in range(MA, MT):
                for ni in range(NT):
                    ps = pp.tile([128, NCH], f32, tag="ps")
                    for kg in range(KG):
                        nc.tensor.matmul(
                            ps[:],
                            ims[kg][:, :, mi * 128:(mi + 1) * 128],
                            sts[kg][:, :, ni * NCH:(ni + 1) * NCH],
                            start=(kg == 0),
                            stop=(kg == KG - 1),
                            perf_mode=dr,
                        )
                    finish_group(mi, ni, ps)
    nc.compile()
    return nc


def _pack_fp8(xT):
    """xT [1024, C] fp8 -> [KG, 128, 2, C] with [kg,p,i,c] = xT[kg*256+i*128+p, c]."""
    C = xT.shape[1]
    return np.ascontiguousarray(
        xT.reshape(KG, 2, 128, C).transpose(0, 2, 1, 3))


def _run_device(s_np, im_np):
    """Returns g4 [B, B, L, R] fp32: g4[i,j,w,r] = s[i,w] . im[j,r]."""
    global LAST_RESULTS
    import ml_dtypes
    from concourse import bass_utils

    if "nc" not in _CACHE:
        _CACHE["nc"] = _build_bass()
    nc = _CACHE["nc"]

    f8 = ml_dtypes.float8_e4m3
    s8 = s_np.astype(f8)
    im8 = im_np.astype(f8)
    in_maps = []
    for c in range(N_CORES):
        ti, tj = c // TJ, c % TJ
        sblk = s8[ti * BT_LOC:(ti + 1) * BT_LOC].reshape(N, K)
        iblk = im8[tj * BI_LOC:(tj + 1) * BI_LOC].reshape(M, K)
        in_maps.append({
            "sT8": _pack_fp8(np.ascontiguousarray(sblk.T)),
            "imT8": _pack_fp8(np.ascontiguousarray(iblk.T)),
        })
    res = bass_utils.run_bass_kernel_spmd(
        nc, in_maps, core_ids=list(range(N_CORES)),
        trace=bool(os.environ.get("KERNEL_TRACE")),
    )
    LAST_RESULTS = res
    g4 = np.empty((B, B, L, R), dtype=np.float32)
    for c in range(N_CORES):
        ti, tj = c // TJ, c % TJ
        gb = res.results[c]["gT"].astype(np.float32)       # [1152, 800]
        blk = gb.reshape(BI_LOC, R, BT_LOC, L).transpose(2, 0, 3, 1)
        g4[ti * BT_LOC:(ti + 1) * BT_LOC,
           tj * BI_LOC:(tj + 1) * BI_LOC] = blk
    return g4


def _host_finish(g4, im, s, img_ent, cap_ent, cap_lens):
    f32 = np.float32
    w_idx = np.arange(L)
    word_valid = w_idx[None, :] < cap_lens[:, None]             # [Bt, L]

    attn = np.where(g4 > 0, g4, f32(0.1) * g4)
    attn = attn * word_valid[:, None, :, None].astype(f32)
    attn = attn / (np.sqrt(np.sum(attn * attn, axis=2, keepdims=True)) + f32(EPS))
    z = attn * f32(LAMBDA_SOFTMAX)
    z = z - z.max(axis=-1, keepdims=True)
    e = np.exp(z)
    a = e / e.sum(axis=-1, keepdims=True)
    a = a * (a > 1.0 / R).astype(f32)

    dot_swc = np.sum(a * g4, axis=-1)                           # [Bt,Bi,L]
    gram = np.einsum("jrd,jqd->jrq", im, im)                    # [Bi,R,R]
    t = np.einsum("ijwr,jrq->ijwq", a, gram, optimize=True)
    wc_sq = np.sum(t * a, axis=-1)
    wc_norm = np.sqrt(np.maximum(wc_sq, f32(1e-24)))
    ns = np.sqrt(np.sum(s * s, axis=-1))                        # [Bt,L]
    cos = dot_swc / np.maximum(ns[:, None, :] * wc_norm, f32(EPS))
    cos = np.where(word_valid[:, None, :], cos, f32(-np.inf))
    srt = np.sort(cos, axis=-1)[..., ::-1]
    k = cap_lens - cap_lens // 3
    keep = w_idx[None, None, :] < k[:, None, None]
    latent = np.where(keep, srt, f32(0.0)).sum(axis=-1) / k[:, None].astype(f32)

    n_min = np.minimum(cap_lens, 50)
    ent_ok = (cap_ent != 0) & (w_idx[None, :] < n_min[:, None])
    match = (cap_ent[:, None, :, None] == img_ent[None, :, None, :]) \
        & ent_ok[:, None, :, None]
    nim = np.sqrt(np.sum(im * im, axis=-1))                     # [Bi,R]
    denom = np.maximum(ns[:, None, :, None] * nim[None, :, None, :], f32(EPS))
    direct = np.where(match, g4 / denom, f32(0.0)).sum(axis=(2, 3)) \
        / n_min[:, None].astype(f32)

    scores = latent + direct                                    # [Bt,Bi]
    diag = np.diag(scores).copy()
    cost_s = np.maximum(f32(MARGIN) + scores - diag[:, None], f32(0.0))
    cost_im = np.maximum(f32(MARGIN) + scores - diag[None, :], f32(0.0))
    np.fill_diagonal(cost_s, 0.0)
    np.fill_diagonal(cost_im, 0.0)
    return np.float32(cost_s.max(axis=1).sum() + cost_im.max(axis=0).sum())


def kernel(im, s, image_entity_idxs, caps_entity_idxs, s_l):
    im = np.asarray(im, dtype=np.float32)
    s = np.asarray(s, dtype=np.float32)
    img_ent = np.asarray(image_entity_idxs)
    cap_ent = np.asarray(caps_entity_idxs)
    cap_lens = np.asarray(s_l)
    g4 = _run_device(s, im)
    return _host_finish(g4, im, s, img_ent, cap_ent, cap_lens)


# revision 13
# speedup vs baseline: 2.6466x; 1.0055x over previous
"""Contrastive loss kernel for Trainium2 (8 NeuronCores).

Strategy: shard the pairwise score computation on a 4x2 grid (4 caption
groups x 2 image groups).  Each core computes its block of the raw pairwise
dot tensor  g[i,w,j,r] = s[i,w,:] . im[j,r,:]  (the dominant 15 GFLOP
contraction over D=1024) on the TensorEngine in fp8-e4m3 with DoubleRow
perf mode (2 MACs/cell/cycle).  Rows of the device matmul are (j,r) pairs
-- 1152 = 9*128 exactly, so no ragged m-tiles -- and columns are the (i,w)
pairs of the local caption group.  PSUM accumulates in fp32 and the g block
is written back in fp16, so the only precision loss is the fp8 input
rounding (~1e-3 on the final loss; tolerance is 2e-2).

DMA plan: few large transfers (descriptor generation costs ~650 ns per
dma_start on the issuing sequencer).  im k-group tiles go on the sync
HWDGE ring, s k-group tiles on the scalar HWDGE ring, outputs on the
gpsimd SWDGE ring.  The first 8 psum groups accumulate k-outer (one k-step
per group as each (im_kg, s_kg) pair lands) so the PE never waits for the
full input set.

The remaining cheap reductions (leaky-relu attention, softmax, top-k word
pooling, entity-matched direct score, margin reduction) run on host in
float32.
"""

import os
import sys

import numpy as np

sys.path.insert(0, "/opt/trn_rl_repo")

B, R, L, D = 64, 36, 50, 1024
N_CORES = 8
TI, TJ = 4, 2                  # caption groups x image groups
BT_LOC = B // TI               # 16 captions per core
BI_LOC = B // TJ               # 32 images per core
M = BI_LOC * R                 # 1152 matmul rows   (j_local, r) = 9 * 128
N = BT_LOC * L                 # 800 matmul cols    (i_local, w)
K = D                          # 1024 contraction
KG = 4                         # k-groups of 256 (2 x 128 for DoubleRow)
NCH = 400                      # psum free-dim chunk (2 chunks of 400)
MA = 4                         # m-tiles in wave 1
LAMBDA_SOFTMAX = 9.0
MARGIN = 0.2
EPS = 1e-8

_CACHE = {}
LAST_RESULTS = None  # BassKernelResults from the most recent run (for test.py)


def _build_bass():
    import concourse.bacc as bacc
    import concourse.mybir as mybir
    import concourse.tile as tile

    nc = bacc.Bacc(
        "TRN2",
        target_bir_lowering=False,
        debug=False,
        enable_asserts=False,
        num_devices=1,
    )
    f8 = mybir.dt.float8e4
    f16 = mybir.dt.float16
    f32 = mybir.dt.float32
    dr = mybir.MatmulPerfMode.DoubleRow
    # [kg, p, i, c]: element = xT[kg*256 + i*128 + p, c]
    sT8 = nc.dram_tensor("sT8", [KG, 128, 2, N], f8, kind="ExternalInput").ap()
    imT8 = nc.dram_tensor("imT8", [KG, 128, 2, M], f8,
                          kind="ExternalInput").ap()
    gT = nc.dram_tensor("gT", [M, N], f16, kind="ExternalOutput").ap()

    MT = M // 128          # 9 row tiles (even)
    NT = N // NCH          # 2 column chunks
    with tile.TileContext(nc) as tc:
        with (
            tc.tile_pool(name="sp", bufs=KG) as sp,
            tc.tile_pool(name="ip", bufs=KG) as ip,
            tc.tile_pool(name="ps", bufs=8, space="PSUM") as pp,
            tc.tile_pool(name="out", bufs=4) as op,
            tc.tile_pool(name="wu", bufs=1) as wp,
        ):
            ims, sts = [], []
            for kg in range(KG):
                it = ip.tile([128, 2, M], f8, tag="im", name=f"im_{kg}")
                nc.sync.dma_start(it[:], imT8[kg])
                ims.append(it)
                st = sp.tile([128, 2, N], f8, tag="s", name=f"s_{kg}")
                nc.scalar.dma_start(st[:], sT8[kg])
                sts.append(st)

            outs = {}

            def finish_group(mi, ni, ps):
                if mi not in outs:
                    outs[mi] = op.tile([128, N], f16, tag="out",
                                       name=f"out_{mi}")
                ot = outs[mi]
                nc.vector.tensor_copy(ot[:, ni * NCH:(ni + 1) * NCH], ps[:])
                if ni == NT - 1:
                    nc.gpsimd.dma_start(
                        gT[mi * 128:(mi + 1) * 128, :], ot[:])

            # Wave 1: 8 psum groups (m 0..3 x n 0..1), k-outer so each
            # (im_kg, s_kg) arrival unlocks one k-step for every open group.
            w1 = [(mi, ni) for mi in range(MA) for ni in range(NT)]
            pss = {g: pp.tile([128, NCH], f32, tag="ps", name=f"ps_{g[0]}_{g[1]}")
                   for g in w1}

            # HAM warm-up: the PE clock-gate needs ~3.4 us of sustained
            # activity to go 1.2 -> 2.4 GHz, and the input DMAs don't land
            # until ~3 us after engine init.  Fill that window with dummy
            # matmuls on a scratch tile (contents discarded).  The scratch
            # psum tile shares the "ps" slots, so it is released before the
            # last wave-1 group needs its bank.
            wut = wp.tile([128, NCH], f16, tag="wu")
            nc.vector.memset(wut[:], 0.0)
            wups = pp.tile([128, NCH], f32, tag="ps", name="wups")
            for _ in range(8):
                nc.tensor.matmul(
                    wups[:], wut[:, 0:128], wut[:],
                    start=True, stop=True)

            for kg in range(KG):
                for (mi, ni) in w1:
                    nc.tensor.matmul(
                        pss[(mi, ni)][:],
                        ims[kg][:, :, mi * 128:(mi + 1) * 128],
                        sts[kg][:, :, ni * NCH:(ni + 1) * NCH],
                        start=(kg == 0),
                        stop=(kg == KG - 1),
                        perf_mode=dr,
                    )
            for (mi, ni) in w1:
                finish_group(mi, ni, pss[(mi, ni)])

            # Wave 2: m-tiles 4..8, data resident by now; k-inner per group.
            # The very last column chunk is sub-split (256+144) so the final
            # cast -> dma -> completion chain after the last matmul is short.
            for mi in range(MA, MT):
                last_m = mi == MT - 1
                chunks = [(0, NCH), (NCH, NCH)] if not last_m else \
                    [(0, NCH), (NCH, 256), (NCH + 256, 144)]
                for ci, (c0, cw) in enumerate(chunks):
                    ps = pp.tile([128, NCH], f32, tag="ps")
                    for kg in range(KG):
                        nc.tensor.matmul(
                            ps[:, :cw],
                            ims[kg][:, :, mi * 128:(mi + 1) * 128],
                            sts[kg][:, :, c0:c0 + cw],
                            start=(kg == 0),
                            stop=(kg == KG - 1),
                            perf_mode=dr,
                        )
                    if mi not in outs:
                        outs[mi] = op.tile([128, N], f16, tag="out",
                                           name=f"out_{mi}")
                    ot = outs[mi]
                    nc.vector.tensor_copy(ot[:, c0:c0 + cw], ps[:, :cw])
                    if last_m:
                        nc.scalar.dma_start(
                            gT[mi * 128:(mi + 1) * 128, c0:c0 + cw],
                            ot[:, c0:c0 + cw])
                    elif ci == len(chunks) - 1:
                        nc.gpsimd.dma_start(
                            gT[mi * 128:(mi + 1) * 128, :], ot[:])
    nc.compile()
    return nc


def _pack_fp8(xT):
    """xT [1024, C] fp8 -> [KG, 128, 2, C] with [kg,p,i,c] = xT[kg*256+i*128+p, c]."""
    C = xT.shape[1]
    return np.ascontiguousarray(
        xT.reshape(KG, 2, 128, C).transpose(0, 2, 1, 3))


def _run_device(s_np, im_np):
    """Returns g4 [B, B, L, R] fp32: g4[i,j,w,r] = s[i,w] . im[j,r]."""
    global LAST_RESULTS
    import ml_dtypes
    from concourse import bass_utils

    if "nc" not in _CACHE:
        _CACHE["nc"] = _build_bass()
    nc = _CACHE["nc"]

    f8 = ml_dtypes.float8_e4m3
    s8 = s_np.astype(f8)
    im8 = im_np.astype(f8)
    in_maps = []
    for c in range(N_CORES):
        ti, tj = c // TJ, c % TJ
        sblk = s8[ti * BT_LOC:(ti + 1) * BT_LOC].reshape(N, K)
        iblk = im8[tj * BI_LOC:(tj + 1) * BI_LOC].reshape(M, K)
        in_maps.append({
            "sT8": _pack_fp8(np.ascontiguousarray(sblk.T)),
            "imT8": _pack_fp8(np.ascontiguousarray(iblk.T)),
        })
    res = bass_utils.run_bass_kernel_spmd(
        nc, in_maps, core_ids=list(range(N_CORES)),
        trace=bool(os.environ.get("KERNEL_TRACE")),
    )
    LAST_RESULTS = res
    g4 = np.empty((B, B, L, R), dtype=np.float32)
    for c in range(N_CORES):
        ti, tj = c // TJ, c % TJ
        gb = res.results[c]["gT"].astype(np.float32)       # [1152, 800]
        blk = gb.reshape(BI_LOC, R, BT_LOC, L).transpose(2, 0, 3, 1)
        g4[ti * BT_LOC:(ti + 1) * BT_LOC,
           tj * BI_LOC:(tj + 1) * BI_LOC] = blk
    return g4


def _host_finish(g4, im, s, img_ent, cap_ent, cap_lens):
    f32 = np.float32
    w_idx = np.arange(L)
    word_valid = w_idx[None, :] < cap_lens[:, None]             # [Bt, L]

    attn = np.where(g4 > 0, g4, f32(0.1) * g4)
    attn = attn * word_valid[:, None, :, None].astype(f32)
    attn = attn / (np.sqrt(np.sum(attn * attn, axis=2, keepdims=True)) + f32(EPS))
    z = attn * f32(LAMBDA_SOFTMAX)
    z = z - z.max(axis=-1, keepdims=True)
    e = np.exp(z)
    a = e / e.sum(axis=-1, keepdims=True)
    a = a * (a > 1.0 / R).astype(f32)

    dot_swc = np.sum(a * g4, axis=-1)                           # [Bt,Bi,L]
    gram = np.einsum("jrd,jqd->jrq", im, im)                    # [Bi,R,R]
    t = np.einsum("ijwr,jrq->ijwq", a, gram, optimize=True)
    wc_sq = np.sum(t * a, axis=-1)
    wc_norm = np.sqrt(np.maximum(wc_sq, f32(1e-24)))
    ns = np.sqrt(np.sum(s * s, axis=-1))                        # [Bt,L]
    cos = dot_swc / np.maximum(ns[:, None, :] * wc_norm, f32(EPS))
    cos = np.where(word_valid[:, None, :], cos, f32(-np.inf))
    srt = np.sort(cos, axis=-1)[..., ::-1]
    k = cap_lens - cap_lens // 3
    keep = w_idx[None, None, :] < k[:, None, None]
    latent = np.where(keep, srt, f32(0.0)).sum(axis=-1) / k[:, None].astype(f32)

    n_min = np.minimum(cap_lens, 50)
    ent_ok = (cap_ent != 0) & (w_idx[None, :] < n_min[:, None])
    match = (cap_ent[:, None, :, None] == img_ent[None, :, None, :]) \
        & ent_ok[:, None, :, None]
    nim = np.sqrt(np.sum(im * im, axis=-1))                     # [Bi,R]
    denom = np.maximum(ns[:, None, :, None] * nim[None, :, None, :], f32(EPS))
    direct = np.where(match, g4 / denom, f32(0.0)).sum(axis=(2, 3)) \
        / n_min[:, None].astype(f32)

    scores = latent + direct                                    # [Bt,Bi]
    diag = np.diag(scores).copy()
    cost_s = np.maximum(f32(MARGIN) + scores - diag[:, None], f32(0.0))
    cost_im = np.maximum(f32(MARGIN) + scores - diag[None, :], f32(0.0))
    np.fill_diagonal(cost_s, 0.0)
    np.fill_diagonal(cost_im, 0.0)
    return np.float32(cost_s.max(axis=1).sum() + cost_im.max(axis=0).sum())


def kernel(im, s, image_entity_idxs, caps_entity_idxs, s_l):
    im = np.asarray(im, dtype=np.float32)
    s = np.asarray(s, dtype=np.float32)
    img_ent = np.asarray(image_entity_idxs)
    cap_ent = np.asarray(caps_entity_idxs)
    cap_lens = np.asarray(s_l)
    g4 = _run_device(s, im)
    return _host_finish(g4, im, s, img_ent, cap_ent, cap_lens)
